# revision 4
# baseline (speedup 1.0000x reference)
"""Distributed Trainium2 kernel for nn_ContrastiveLoss (SimCLR InfoNCE loss).

fp8e4 DoubleRow + symmetry with staircase diagonal blocks.

C = zhat zhat^T is symmetric. Rows are rolled per core so core c owns
slab c; block distance d = (col_block - c) mod 8. Each core computes:

  d=0 (own slab):  per m-tile, the 128-col staircase [m*128, 1024) —
      the 128x128 diagonal tile fully (row-sums only, self-diag
      extracted) plus the strictly-upper tiles (row-sums AND col-sums).
  d=1..3: full 1024-col blocks (row-sums AND col-sums).
  d=4:  same staircase as d=0 shifted to cols [4096+m*128, 5120); its
      diagonal tile holds the positive pairs (row-sums only).

Every unordered pair is computed exactly once; each exp'd element
feeds its row's softmax denominator via ACT accum row-sums and its
column's via DVE-accumulated column partials. colacc is laid out as 5
regions of 1024 local columns (d = 0..4); region d holds partial sums
destined for slab (c+d) mod 8. The host gathers row/column partials,
raw diag/pos dots, and finishes in fp64 (log, combine, mean).

PE: 288 DoubleRow matmuls totaling 33792 output columns per core
(vs 65536 for the dense slab) — about 56 us of stream time at 2.4 GHz.
"""

import numpy as np

N, D = 8192, 1024
NCORES = 8
ROWS = N // NCORES
MT = ROWS // 128
KK = 4
NTILE = 512
NLOC = 10 * NTILE       # local cols used: 0..5119
SCALE = 256.0
EFF = 10.0 / (SCALE * SCALE)


def _segs(m):
    """Per m-tile: list of groups; each group is a list of segments
    (local_col_base, width, kind) with kind in {'nt','d0','d4'}.
    Variable-width staircase segments go last so the PSUM layout is
    contiguous and every matmul chunk stays 512-bank-aligned."""
    w = 1024 - m * 128
    d0 = (m * 128, w, "d0")
    d4 = (4096 + m * 128, w, "d4")
    nt = lambda t: (t * NTILE, NTILE, "nt")
    if m < 4:
        return [
            [nt(2), nt(3), d0],
            [nt(4), nt(5), nt(6), nt(7)],
            [d4],
        ]
    return [
        [nt(2), nt(3), nt(4), d0],
        [nt(5), nt(6), nt(7), d4],
    ]


def _import_concourse():
    import sys
    try:
        import concourse.bass  # noqa: F401
    except ImportError:
        for p in ("/root/.axon_site/_ro/trn_rl_repo", "/opt/trn_rl_repo"):
            if p not in sys.path:
                sys.path.insert(0, p)
        import concourse.bass  # noqa: F401


def _sched():
    out = []
    for m in range(MT):
        out.append((m, 0))
    for m in range(MT):
        out.append((m, 1))
    for m in range(4):
        out.append((m, 2))
    return out


def build_program():
    _import_concourse()
    import concourse.mybir as mybir
    import concourse.tile as tile
    from concourse import bacc
    from concourse.masks import make_identity

    f32 = mybir.dt.float32
    f8 = mybir.dt.float8e4
    Act = mybir.ActivationFunctionType
    DR = mybir.MatmulPerfMode.DoubleRow

    sched = _sched()
    accidx = {mg: i for i, mg in enumerate(sched)}
    nacc = len(sched)

    nc = bacc.Bacc()
    zq = nc.declare_dram_parameter("zq", [128, KK * 2 * NLOC], f8, isOutput=False)
    acc_d = nc.declare_dram_parameter("acc", [128, nacc], f32, isOutput=True)
    ddiag_d = nc.declare_dram_parameter("ddiag", [128, MT * 128], f32, isOutput=True)
    pdiag_d = nc.declare_dram_parameter("pdiag", [128, MT * 128], f32, isOutput=True)
    colacc_d = nc.declare_dram_parameter(
        "colacc", [128, 5 * 1024], f32, isOutput=True
    )

    zq_d = zq.rearrange("p (kk i n) -> p kk i n", kk=KK, i=2)

    with tile.TileContext(nc) as tc:
        with (
            tc.tile_pool(name="consts", bufs=1) as consts,
            tc.tile_pool(name="zqp", bufs=1) as zqp,
            tc.tile_pool(name="psump", bufs=2, space="PSUM") as psump,
            tc.tile_pool(name="escp", bufs=3) as escp,
            tc.tile_pool(name="smallp", bufs=4) as smallp,
            tc.tile_pool(name="accp", bufs=1) as accp,
        ):
            ident = consts.tile([128, 128], f32)
            make_identity(nc, ident)
            identw = consts.tile([128, 1], f32)
            nc.vector.reduce_max(
                out=identw, in_=ident, axis=mybir.AxisListType.X
            )
            # ACT exp-table warm-up during the input-DMA window
            actwarm = consts.tile([128, 1], f32)
            nc.scalar.activation(
                out=actwarm, in_=identw, func=Act.Exp, scale=0.001
            )

            acc = accp.tile([128, nacc], f32)
            colacc = accp.tile([128, 5 * 1024], f32)
            nc.any.memset(colacc, 0)

            zq_t = zqp.tile([128, KK, 2, NLOC], f8)
            for a, b in ((0, 2048), (2048, 2560), (2560, NLOC)):
                for kk in range(KK):
                    nc.sync.dma_start(
                        out=zq_t[:, kk, :, a:b],
                        in_=zq_d[:, kk, :, a:b],
                    )

            # PE warm-up on a zeroed tile while the input DMA lands:
            # HAM starts at K=4/8 (1.2 GHz) and needs ~3.4us of sustained
            # activity, so the first real matmuls run at full clock.
            warm8 = consts.tile([128, 2, 512], f8)
            nc.any.memset(warm8, 0)
            warmps = psump.tile([128, 4 * NTILE], f32, name="ps")
            for _ in range(12):
                nc.tensor.matmul(
                    warmps[:, :512],
                    lhsT=warm8[:, :, :128],
                    rhs=warm8,
                    start=True,
                    stop=True,
                    perf_mode=DR,
                )

            for m, gi in sched:
                g = _segs(m)[gi]
                gw = sum(s[1] for s in g)
                ps4 = psump.tile([128, 4 * NTILE], f32, name="ps")
                for kk in range(KK):
                    lhsT = zq_t[:, kk, :, m * 128 : (m + 1) * 128]
                    off = 0
                    for base, w, _kind in g:
                        done = 0
                        while done < w:
                            cw = min(NTILE, w - done)
                            nc.tensor.matmul(
                                ps4[:, off + done : off + done + cw],
                                lhsT=lhsT,
                                rhs=zq_t[
                                    :, kk, :, base + done : base + done + cw
                                ],
                                start=(kk == 0),
                                stop=(kk == KK - 1),
                                perf_mode=DR,
                            )
                            done += cw
                        off += w
                esc = escp.tile([128, 4 * NTILE], f32)
                ai = accidx[(m, gi)]
                nc.scalar.activation(
                    out=esc[:, :gw],
                    in_=ps4[:, :gw],
                    func=Act.Exp,
                    scale=EFF,
                    accum_out=acc[:, ai : ai + 1],
                )
                # exp'd diagonal tiles out via DMA; host extracts the
                # diagonal and takes logs (saves DVE work on the PSUM path)
                off = 0
                for base, w, kind in g:
                    if kind == "d0":
                        nc.sync.dma_start(
                            out=ddiag_d[:, m * 128 : (m + 1) * 128],
                            in_=esc[:, off : off + 128],
                        )
                    elif kind == "d4":
                        nc.sync.dma_start(
                            out=pdiag_d[:, m * 128 : (m + 1) * 128],
                            in_=esc[:, off : off + 128],
                        )
                    off += w
                # column partial sums (skip the 128-wide diagonal tiles)
                off = 0
                for base, w, kind in g:
                    if kind == "nt":
                        # colacc region d*1024 + block-local col == base
                        src = esc[:, off : off + w]
                        dst = colacc[:, base : base + w]
                        nc.vector.tensor_add(out=dst, in0=dst, in1=src)
                    else:
                        # staircase: cols after the first 128 (diag tile)
                        if w > 128:
                            src = esc[:, off + 128 : off + w]
                            reg = 0 if kind == "d0" else 4
                            lo = reg * 1024 + m * 128 + 128
                            dst = colacc[:, lo : lo + w - 128]
                            nc.vector.tensor_add(out=dst, in0=dst, in1=src)
                    off += w
                # flush outputs whose last writer just ran
                if (m, gi) == (MT - 1, 0):
                    nc.sync.dma_start(out=acc_d[:, :MT], in_=acc[:, :MT])
                    # regions 0 (own) and 1 (nt2,nt3) complete
                    nc.sync.dma_start(
                        out=colacc_d[:, :2048], in_=colacc[:, :2048]
                    )
                if (m, gi) == (2, 2):
                    nc.sync.dma_start(
                        out=colacc_d[:, 4096:4608], in_=colacc[:, 4096:4608]
                    )
                    nc.sync.dma_start(
                        out=acc_d[:, 16:19], in_=acc[:, 16:19]
                    )
                if (m, gi) == (MT - 1, 1):
                    nc.sync.dma_start(
                        out=acc_d[:, MT : 2 * MT], in_=acc[:, MT : 2 * MT]
                    )
                    # regions 2,3 (nt4..nt7) complete
                    nc.sync.dma_start(
                        out=colacc_d[:, 2048:4096], in_=colacc[:, 2048:4096]
                    )

            nc.sync.dma_start(out=acc_d[:, 19:], in_=acc[:, 19:])
            nc.sync.dma_start(out=colacc_d[:, 4608:], in_=colacc[:, 4608:])
    nc.finalize()
    return nc


def make_in_maps(z: np.ndarray) -> list[dict]:
    import ml_dtypes

    z = np.ascontiguousarray(np.asarray(z, dtype=np.float32))
    norms = np.sqrt((z.astype(np.float64) ** 2).sum(axis=-1))
    zn = (z / norms[:, None]).astype(np.float32)
    q = (zn * np.float32(SCALE)).astype(ml_dtypes.float8_e4m3)  # [N, D]
    qt = np.ascontiguousarray(q.T)  # [D, N]
    qr = qt.reshape(KK, 2, 128, N).transpose(2, 0, 1, 3)  # [p, kk, i, n]
    in_maps = []
    for c in range(NCORES):
        s = c * ROWS
        zc = np.concatenate([qr[..., s:], qr[..., :s]], axis=-1) if s else qr
        zc = zc[..., :NLOC]
        in_maps.append({"zq": np.ascontiguousarray(zc.reshape(128, -1))})
    return in_maps


def assemble(results: list[dict]) -> np.ndarray:
    sched = _sched()
    accidx = {mg: i for i, mg in enumerate(sched)}
    ngroups = {m: len(_segs(m)) for m in range(MT)}

    S = np.zeros(N, np.float64)
    pvals = np.zeros(N, np.float64)

    for c, r in enumerate(results):
        acc = np.asarray(r["acc"], np.float64)
        ddiag = np.asarray(r["ddiag"], np.float64)
        pdiag = np.asarray(r["pdiag"], np.float64)
        colacc = np.asarray(r["colacc"], np.float64)
        pidx = np.arange(128)

        base = c * ROWS
        for m in range(MT):
            rows = base + m * 128 + np.arange(128)
            tot = np.zeros(128, np.float64)
            for gi in range(ngroups[m]):
                tot += acc[:, accidx[(m, gi)]]
            # self term was counted once (row-sums only); ddiag holds
            # exp(EFF*selfdot) directly
            tot -= ddiag[pidx, m * 128 + pidx]
            S[rows] += tot
            pvals[rows] = pdiag[pidx, m * 128 + pidx]

        csum = colacc.sum(axis=0)  # [5120]
        for d in range(5):
            dest = ((c + d) % NCORES) * ROWS + np.arange(1024)
            S[dest] += csum[d * 1024 : (d + 1) * 1024]

    nll = np.log(S) - np.log(pvals)  # pvals = exp(EFF*posdot)
    return np.float32(nll.mean())


def kernel(z: np.ndarray) -> np.ndarray:
    _import_concourse()
    from concourse.bass_utils import run_bass_kernel_spmd

    nc = build_program()
    in_maps = make_in_maps(z)
    res = run_bass_kernel_spmd(nc, in_maps, core_ids=list(range(NCORES)))
    return assemble(res.results)


# revision 5
# speedup vs baseline: 1.0564x; 1.0564x over previous
"""Distributed Trainium2 kernel for nn_ContrastiveLoss (SimCLR InfoNCE loss).

fp8e4 DoubleRow + symmetry with staircase diagonal blocks.

C = zhat zhat^T is symmetric. Rows are rolled per core so core c owns
slab c; block distance d = (col_block - c) mod 8. Each core computes:

  d=0 (own slab):  per m-tile, the 128-col staircase [m*128, 1024) —
      the 128x128 diagonal tile fully (row-sums only, self-diag
      extracted) plus the strictly-upper tiles (row-sums AND col-sums).
  d=1..3: full 1024-col blocks (row-sums AND col-sums).
  d=4:  same staircase as d=0 shifted to cols [4096+m*128, 5120); its
      diagonal tile holds the positive pairs (row-sums only).

Every unordered pair is computed exactly once; each exp'd element
feeds its row's softmax denominator via ACT accum row-sums and its
column's via DVE-accumulated column partials. colacc is laid out as 5
regions of 1024 local columns (d = 0..4); region d holds partial sums
destined for slab (c+d) mod 8. The host gathers row/column partials,
raw diag/pos dots, and finishes in fp64 (log, combine, mean).

PE: 288 DoubleRow matmuls totaling 33792 output columns per core
(vs 65536 for the dense slab) — about 56 us of stream time at 2.4 GHz.
"""

import numpy as np

N, D = 8192, 1024
NCORES = 8
ROWS = N // NCORES
MT = ROWS // 128
KK = 4
NTILE = 512
NLOC = 10 * NTILE       # local cols used: 0..5119
SCALE = 256.0
EFF = 10.0 / (SCALE * SCALE)


def _segs(m):
    """Per m-tile: list of groups; each group is a list of segments
    (local_col_base, width, kind) with kind in {'nt','d0','d4'}.
    Variable-width staircase segments go last so the PSUM layout is
    contiguous and every matmul chunk stays 512-bank-aligned."""
    w = 1024 - m * 128
    d0 = (m * 128, w, "d0")
    d4 = (4096 + m * 128, w, "d4")
    nt = lambda t: (t * NTILE, NTILE, "nt")
    if m < 4:
        return [
            [nt(2), nt(3), d0],
            [nt(4), nt(5), nt(6), nt(7)],
            [d4],
        ]
    return [
        [nt(2), nt(3), nt(4), d0],
        [nt(5), nt(6), nt(7), d4],
    ]


def _import_concourse():
    import sys
    try:
        import concourse.bass  # noqa: F401
    except ImportError:
        for p in ("/root/.axon_site/_ro/trn_rl_repo", "/opt/trn_rl_repo"):
            if p not in sys.path:
                sys.path.insert(0, p)
        import concourse.bass  # noqa: F401


def _sched():
    out = []
    for m in range(MT):
        out.append((m, 0))
    for m in range(MT):
        out.append((m, 1))
    for m in range(4):
        out.append((m, 2))
    return out


def build_program():
    _import_concourse()
    import concourse.mybir as mybir
    import concourse.tile as tile
    from concourse import bacc
    from concourse.masks import make_identity

    f32 = mybir.dt.float32
    f8 = mybir.dt.float8e4
    Act = mybir.ActivationFunctionType
    DR = mybir.MatmulPerfMode.DoubleRow

    sched = _sched()
    accidx = {mg: i for i, mg in enumerate(sched)}
    nacc = len(sched)

    nc = bacc.Bacc()
    zq = nc.declare_dram_parameter("zq", [128, KK * 2 * NLOC], f8, isOutput=False)
    acc_d = nc.declare_dram_parameter("acc", [128, nacc], f32, isOutput=True)
    ddiag_d = nc.declare_dram_parameter("ddiag", [128, MT * 128], f32, isOutput=True)
    pdiag_d = nc.declare_dram_parameter("pdiag", [128, MT * 128], f32, isOutput=True)
    colacc_d = nc.declare_dram_parameter(
        "colacc", [128, 5 * 1024], f32, isOutput=True
    )

    zq_d = zq.rearrange("p (kk i n) -> p kk i n", kk=KK, i=2)

    with tile.TileContext(nc) as tc:
        with (
            tc.tile_pool(name="consts", bufs=1) as consts,
            tc.tile_pool(name="zqp", bufs=1) as zqp,
            tc.tile_pool(name="psump", bufs=2, space="PSUM") as psump,
            tc.tile_pool(name="escp", bufs=3) as escp,
            tc.tile_pool(name="smallp", bufs=4) as smallp,
            tc.tile_pool(name="accp", bufs=1) as accp,
        ):
            ident = consts.tile([128, 128], f32)
            make_identity(nc, ident)
            identw = consts.tile([128, 1], f32)
            nc.vector.reduce_max(
                out=identw, in_=ident, axis=mybir.AxisListType.X
            )
            # ACT exp-table warm-up during the input-DMA window
            actwarm = consts.tile([128, 1], f32)
            nc.scalar.activation(
                out=actwarm, in_=identw, func=Act.Exp, scale=0.001
            )

            acc = accp.tile([128, nacc], f32)
            colacc = accp.tile([128, 5 * 1024], f32)
            nc.any.memset(colacc, 0)

            zq_t = zqp.tile([128, KK, 2, NLOC], f8)
            for a, b in ((0, 2048), (2048, 2560), (2560, NLOC)):
                for kk in range(KK):
                    nc.sync.dma_start(
                        out=zq_t[:, kk, :, a:b],
                        in_=zq_d[:, kk, :, a:b],
                    )

            # PE warm-up on a zeroed tile while the input DMA lands:
            # HAM starts at K=4/8 (1.2 GHz) and needs ~3.4us of sustained
            # activity, so the first real matmuls run at full clock.
            warm8 = consts.tile([128, 2, 512], f8)
            nc.any.memset(warm8, 0)
            warmps = psump.tile([128, 4 * NTILE], f32, name="ps")
            for _ in range(12):
                nc.tensor.matmul(
                    warmps[:, :512],
                    lhsT=warm8[:, :, :128],
                    rhs=warm8,
                    start=True,
                    stop=True,
                    perf_mode=DR,
                )

            for m, gi in sched:
                g = _segs(m)[gi]
                gw = sum(s[1] for s in g)
                ps4 = psump.tile([128, 4 * NTILE], f32, name="ps")
                for kk in range(KK):
                    lhsT = zq_t[:, kk, :, m * 128 : (m + 1) * 128]
                    off = 0
                    for base, w, _kind in g:
                        done = 0
                        while done < w:
                            cw = min(NTILE, w - done)
                            nc.tensor.matmul(
                                ps4[:, off + done : off + done + cw],
                                lhsT=lhsT,
                                rhs=zq_t[
                                    :, kk, :, base + done : base + done + cw
                                ],
                                start=(kk == 0),
                                stop=(kk == KK - 1),
                                perf_mode=DR,
                            )
                            done += cw
                        off += w
                esc = escp.tile([128, 4 * NTILE], f32)
                ai = accidx[(m, gi)]
                nc.scalar.activation(
                    out=esc[:, :gw],
                    in_=ps4[:, :gw],
                    func=Act.Exp,
                    scale=EFF,
                    accum_out=acc[:, ai : ai + 1],
                )
                # exp'd diagonal tiles out via DMA; host extracts the
                # diagonal and takes logs (saves DVE work on the PSUM path)
                off = 0
                for base, w, kind in g:
                    if kind == "d0":
                        nc.sync.dma_start(
                            out=ddiag_d[:, m * 128 : (m + 1) * 128],
                            in_=esc[:, off : off + 128],
                        )
                    elif kind == "d4":
                        nc.sync.dma_start(
                            out=pdiag_d[:, m * 128 : (m + 1) * 128],
                            in_=esc[:, off : off + 128],
                        )
                    off += w
                # column partial sums. The nt segments of a group are
                # contiguous in esc AND in colacc (dest col == local col),
                # so they merge into one wide DVE add; the staircase strip
                # (minus its 128-wide diagonal tile) adds separately.
                off = 0
                nt_lo = nt_hi = None
                for base, w, kind in g:
                    if kind == "nt":
                        if nt_lo is None:
                            nt_lo = (off, base)
                        nt_hi = (off + w, base + w)
                    else:
                        if w > 128:
                            srcv = esc[:, off + 128 : off + w]
                            reg = 0 if kind == "d0" else 4
                            lo = reg * 1024 + m * 128 + 128
                            dst = colacc[:, lo : lo + w - 128]
                            nc.vector.tensor_add(out=dst, in0=dst, in1=srcv)
                    off += w
                if nt_lo is not None:
                    srcv = esc[:, nt_lo[0] : nt_hi[0]]
                    dst = colacc[:, nt_lo[1] : nt_hi[1]]
                    nc.vector.tensor_add(out=dst, in0=dst, in1=srcv)
                # flush outputs whose last writer just ran
                if (m, gi) == (MT - 1, 0):
                    nc.sync.dma_start(out=acc_d[:, :MT], in_=acc[:, :MT])
                    # regions 0 (own) and 1 (nt2,nt3) complete
                    nc.sync.dma_start(
                        out=colacc_d[:, :2048], in_=colacc[:, :2048]
                    )
                if (m, gi) == (2, 2):
                    nc.sync.dma_start(
                        out=colacc_d[:, 4096:4608], in_=colacc[:, 4096:4608]
                    )
                    nc.sync.dma_start(
                        out=acc_d[:, 16:19], in_=acc[:, 16:19]
                    )
                if (m, gi) == (MT - 1, 1):
                    nc.sync.dma_start(
                        out=acc_d[:, MT : 2 * MT], in_=acc[:, MT : 2 * MT]
                    )
                    # regions 2,3 (nt4..nt7) complete
                    nc.sync.dma_start(
                        out=colacc_d[:, 2048:4096], in_=colacc[:, 2048:4096]
                    )

            nc.sync.dma_start(out=acc_d[:, 19:], in_=acc[:, 19:])
            nc.sync.dma_start(out=colacc_d[:, 4608:], in_=colacc[:, 4608:])
    nc.finalize()
    return nc


def make_in_maps(z: np.ndarray) -> list[dict]:
    import ml_dtypes

    z = np.ascontiguousarray(np.asarray(z, dtype=np.float32))
    norms = np.sqrt((z.astype(np.float64) ** 2).sum(axis=-1))
    zn = (z / norms[:, None]).astype(np.float32)
    q = (zn * np.float32(SCALE)).astype(ml_dtypes.float8_e4m3)  # [N, D]
    qt = np.ascontiguousarray(q.T)  # [D, N]
    qr = qt.reshape(KK, 2, 128, N).transpose(2, 0, 1, 3)  # [p, kk, i, n]
    in_maps = []
    for c in range(NCORES):
        s = c * ROWS
        zc = np.concatenate([qr[..., s:], qr[..., :s]], axis=-1) if s else qr
        zc = zc[..., :NLOC]
        in_maps.append({"zq": np.ascontiguousarray(zc.reshape(128, -1))})
    return in_maps


def assemble(results: list[dict]) -> np.ndarray:
    sched = _sched()
    accidx = {mg: i for i, mg in enumerate(sched)}
    ngroups = {m: len(_segs(m)) for m in range(MT)}

    S = np.zeros(N, np.float64)
    pvals = np.zeros(N, np.float64)

    for c, r in enumerate(results):
        acc = np.asarray(r["acc"], np.float64)
        ddiag = np.asarray(r["ddiag"], np.float64)
        pdiag = np.asarray(r["pdiag"], np.float64)
        colacc = np.asarray(r["colacc"], np.float64)
        pidx = np.arange(128)

        base = c * ROWS
        for m in range(MT):
            rows = base + m * 128 + np.arange(128)
            tot = np.zeros(128, np.float64)
            for gi in range(ngroups[m]):
                tot += acc[:, accidx[(m, gi)]]
            # self term was counted once (row-sums only); ddiag holds
            # exp(EFF*selfdot) directly
            tot -= ddiag[pidx, m * 128 + pidx]
            S[rows] += tot
            pvals[rows] = pdiag[pidx, m * 128 + pidx]

        csum = colacc.sum(axis=0)  # [5120]
        for d in range(5):
            dest = ((c + d) % NCORES) * ROWS + np.arange(1024)
            S[dest] += csum[d * 1024 : (d + 1) * 1024]

    nll = np.log(S) - np.log(pvals)  # pvals = exp(EFF*posdot)
    return np.float32(nll.mean())


def kernel(z: np.ndarray) -> np.ndarray:
    _import_concourse()
    from concourse.bass_utils import run_bass_kernel_spmd

    nc = build_program()
    in_maps = make_in_maps(z)
    res = run_bass_kernel_spmd(nc, in_maps, core_ids=list(range(NCORES)))
    return assemble(res.results)
